# revision 12
# baseline (speedup 1.0000x reference)
"""LSTM decoder (nn_Decoder) on 8 Trainium2 NeuronCores.

Strategy (v2):
  - Replicate the sequential LSTM recurrence on all 8 cores (serial in T;
    B=32 gives too little parallelism to shard), shard the output head over
    the vocab dim: core c computes logits[:, :, c*4000:(c+1)*4000].
    Unshard = host-side concat; no device collectives.
  - Input-side gate projections x_gates = e @ W_ih^T + b_ih + b_hh are
    precomputed on HOST (17 GFLOP of the 320 total) and DMA-streamed to SBUF
    as bf16 in the PSUM gate layout. Each step they are injected into the
    PSUM accumulation with 32x32-identity matmuls at tile_position
    (32j, 32j) -- 2 matmul slots/step instead of the 10 the on-device
    embedding projection + rank-1 bias used to take.
  - Recurrence matmuls: gates[32,4096] via stationary h^T [128,32]
    replicated into the PE array's 4 column groups (tile_position=(0,32j)).
    Gate layout on PSUM: partition = 32j+b, free = gate*256+u.
  - h returned to h^T layout by 2 PE transposes into a 4-slot rolling ring
    (only 16 steps of h^T retained), freeing SBUF so that ALL head weights
    (8 vocab tiles x 8 K x 500) stay resident.
  - The vocab head is INTERLEAVED into the recurrence: token tile m
    (=steps 4m..4m+3, 128 tokens) is consumed during steps 4m+4..4m+7,
    one nt-pair (2 x 8 K matmuls, N=500) per step, filling the PE gaps
    left by the elementwise tail. Only tile 31 spills past the loop.
  - All matmuls bf16 inputs / fp32 PSUM; c, gates elementwise in fp32.

Host does data movement / layout prep, the h0 projection, the embedding
gather and the x_gates projection.
"""

import numpy as np
import ml_dtypes
from contextlib import ExitStack

import concourse.bass as bass  # noqa: F401
import concourse.tile as tile
import concourse.bacc as bacc
import concourse.mybir as mybir
from concourse import bass_utils

BF16 = ml_dtypes.bfloat16
N_CORES = 8
B, T = 32, 128
VOCAB, D_EMB, Z_DIM, HID = 32000, 512, 256, 1024
VSH = VOCAB // N_CORES    # 4000 vocab per core
NTOK = B * T              # 4096 tokens; token index = t*32 + b
KH = HID // 128           # 8 hidden K-chunks
GW = 4 * HID              # 4096 gate width
NT_HEAD = 8               # vocab tiles per core in the head
NV = VSH // NT_HEAD       # 500
MT_HEAD = NTOK // 128     # 32 token tiles in the head
CH = 8                    # steps per x_gates prefetch chunk
KORD = (0, 2, 4, 6, 1, 3, 5, 7)  # even h^T chunks first (they land first)

_NC_CACHE = {}


def _perm():
    # psum gate order n = j*1024 + gate*256 + u  ->  torch W column gate*1024 + j*256 + u
    j = np.arange(4)[:, None, None]
    gate = np.arange(4)[None, :, None]
    u = np.arange(256)[None, None, :]
    return (gate * 1024 + j * 256 + u).reshape(-1)


def _build(repeat=1):
    if repeat in _NC_CACHE:
        return _NC_CACHE[repeat]
    nc = bacc.Bacc("TRN2", debug=False, num_devices=N_CORES)
    dt = mybir.dt
    xg_d = nc.dram_tensor("xg", [128, T * 1024], dt.bfloat16, kind="ExternalInput").ap()
    h0T_d = nc.dram_tensor("h0T", [128, KH * B], dt.bfloat16, kind="ExternalInput").ap()
    Ws_d = nc.dram_tensor("Ws", [128, KH * GW], dt.bfloat16, kind="ExternalInput").ap()
    id_d = nc.dram_tensor("ident", [128, 128], dt.bfloat16, kind="ExternalInput").ap()
    WoT_d = nc.dram_tensor("WoT", [128, NT_HEAD * KH * NV], dt.bfloat16, kind="ExternalInput").ap()
    biaso_d = nc.dram_tensor("bias_o", [128, VSH], dt.bfloat16, kind="ExternalInput").ap()
    out_d = nc.dram_tensor("out", [NTOK, VSH], dt.bfloat16, kind="ExternalOutput").ap()

    with tile.TileContext(nc) as tc, ExitStack() as ctx:
        pers = ctx.enter_context(tc.tile_pool(name="pers", bufs=1))
        ident = pers.tile([128, 128], dt.bfloat16)
        nc.sync.dma_start(ident[:], id_d)

        for _rep in range(repeat):
            _emit_body(nc, tc, ident,
                       xg_d, h0T_d, Ws_d, WoT_d, biaso_d, out_d)
    nc.compile()
    _NC_CACHE[repeat] = nc
    return nc


def _emit_body(nc, tc, ident, xg_d, h0T_d, Ws_d, WoT_d, biaso_d, out_d):
    dt = mybir.dt
    ACT = mybir.ActivationFunctionType
    with ExitStack() as rctx:
        wpool = rctx.enter_context(tc.tile_pool(name="ws", bufs=1))
        h0T_s = wpool.tile([128, KH * B], dt.bfloat16)
        nc.sync.dma_start(h0T_s[:], h0T_d)
        ws = wpool.tile([128, KH * GW], dt.bfloat16)    # 64KB/part
        for q in range(4):
            nc.sync.dma_start(ws[:, q * 2 * GW:(q + 1) * 2 * GW],
                              Ws_d[:, q * 2 * GW:(q + 1) * 2 * GW])
        wo = wpool.tile([128, NT_HEAD * KH * NV], dt.bfloat16)  # 64KB/part
        wostride = 2 * KH * NV
        for q in range(4):
            nc.sync.dma_start(wo[:, q * wostride:(q + 1) * wostride],
                              WoT_d[:, q * wostride:(q + 1) * wostride])
        bias_o = wpool.tile([128, VSH], dt.bfloat16)    # 8KB/part
        nc.sync.dma_start(bias_o[:], biaso_d)
        # rolling h^T ring: 4 slots x [KH x 128 tokens]; 8KB/part.
        # chunk index k = 2j+m laid out as (j, m) so the transpose-tail copy
        # can write the 4 even (m=0) / odd (m=1) chunks in one strided op.
        ring = wpool.tile([128, 4 * KH * 128], dt.bfloat16)
        ring_v = ring[:].rearrange("p (s j m tb) -> p s j m tb", s=4, j=4, m=2)

        xgpool = rctx.enter_context(tc.tile_pool(name="xg", bufs=2))
        gpsum = rctx.enter_context(tc.tile_pool(name="gps", bufs=2, space="PSUM"))
        hpsum = rctx.enter_context(tc.tile_pool(name="hps", bufs=2, space="PSUM"))
        tpsum = rctx.enter_context(tc.tile_pool(name="tps", bufs=2, space="PSUM"))
        ew = rctx.enter_context(tc.tile_pool(name="ew", bufs=2))
        opool = rctx.enter_context(tc.tile_pool(name="osb", bufs=4))
        cpool = rctx.enter_context(tc.tile_pool(name="cst", bufs=1))

        c_sb = cpool.tile([128, 256], dt.float32)
        nc.vector.memset(c_sb[:], 0.0)

        def hblock(g, lhsT, k, lo, hi, start, stop):
            for j in range(4):
                nc.tensor.matmul(
                    g[32 * j:32 * j + 32, lo:hi],
                    lhsT,
                    ws[:, k * GW + j * 1024 + lo:k * GW + j * 1024 + hi],
                    start=start, stop=stop,
                    tile_position=(0, 32 * j),
                )

        def head_mms(m, pair, ks, ps0, ps1):
            # head matmuls (K-chunk subset ks) for token tile m, nt-pair
            s = m % 4
            for k in ks:
                lhsT = ring_v[:, s, k // 2, k % 2, :]
                for ps, nt in ((ps0, 2 * pair), (ps1, 2 * pair + 1)):
                    nc.tensor.matmul(
                        ps[:], lhsT,
                        wo[:, (nt * KH + k) * NV:(nt * KH + k + 1) * NV],
                        start=(k == 0), stop=(k == KH - 1),
                    )

        def head_evac(m, pair, ps0, ps1):
            for ps, nt in ((ps0, 2 * pair), (ps1, 2 * pair + 1)):
                osb = opool.tile([128, NV], dt.bfloat16, tag="osb")
                nc.vector.tensor_add(osb[:], ps[:], bias_o[:, nt * NV:(nt + 1) * NV])
                nc.sync.dma_start(
                    out_d[m * 128:(m + 1) * 128, nt * NV:(nt + 1) * NV], osb[:])

        xg_sb = None
        for t in range(T):
            if t % CH == 0:
                xg_sb = xgpool.tile([128, CH * 1024], dt.bfloat16, tag="xg")
                nc.sync.dma_start(xg_sb[:], xg_d[:, t * 1024:(t + CH) * 1024])
            xgt = xg_sb[:, (t % CH) * 1024:(t % CH + 1) * 1024]
            g = gpsum.tile([128, 1024], dt.float32, tag="g")

            def lhs(k):
                if t == 0:
                    return h0T_s[:, k * 32:(k + 1) * 32]
                tp = t - 1
                return ring_v[:, (tp // 4) % 4, k // 2, k % 2,
                              (tp % 4) * 32:(tp % 4) * 32 + 32]

            # region A: i,f gates (psum bank 0); x_gates added on DVE after
            for ki, k in enumerate(KORD):
                hblock(g, lhs(k), k, 0, 512, ki == 0, ki == KH - 1)
            nc.vector.tensor_add(g[:, 0:512], g[:, 0:512], xgt[:, 0:512])
            # region B: g,o gates (psum bank 1)
            for ki, k in enumerate(KORD):
                hblock(g, lhs(k), k, 512, 1024, ki == 0, ki == KH - 1)
            nc.vector.tensor_add(g[:, 512:1024], g[:, 512:1024], xgt[:, 512:1024])

            # interleaved head (tile m = t//4 - 1, nt-pair = t%4): first 6
            # K-chunks cover the elementwise-tail latency, then the h^T
            # transposes slot in exactly when h_bf is ready, then the rest.
            hm = None
            if t >= 4:
                hps0 = hpsum.tile([128, NV], dt.float32, tag="hp", name="hps0")
                hps1 = hpsum.tile([128, NV], dt.float32, tag="hp", name="hps1")
                hm = (t // 4 - 1, t % 4, hps0, hps1)
                head_mms(hm[0], hm[1], range(0, 6), hm[2], hm[3])

            # elementwise tail
            if_sb = ew.tile([128, 512], dt.float32, tag="if")
            nc.scalar.activation(if_sb[:], g[:, 0:512], ACT.Sigmoid)
            gg_sb = ew.tile([128, 256], dt.float32, tag="gg")
            nc.scalar.activation(gg_sb[:], g[:, 512:768], ACT.Tanh)
            o_sb = ew.tile([128, 256], dt.float32, tag="o")
            nc.scalar.activation(o_sb[:], g[:, 768:1024], ACT.Sigmoid)
            nc.vector.tensor_mul(c_sb[:], c_sb[:], if_sb[:, 256:512])
            t1 = ew.tile([128, 256], dt.float32, tag="t1")
            nc.vector.tensor_mul(t1[:], if_sb[:, 0:256], gg_sb[:])
            nc.vector.tensor_add(c_sb[:], c_sb[:], t1[:])
            tc_sb = ew.tile([128, 256], dt.float32, tag="tc")
            nc.scalar.activation(tc_sb[:], c_sb[:], ACT.Tanh)
            h_bf = ew.tile([128, 256], dt.bfloat16, tag="h")
            nc.vector.tensor_mul(h_bf[:], o_sb[:], tc_sb[:])
            s = (t // 4) % 4
            for m in range(2):
                tr = tpsum.tile([128, 128], dt.bfloat16, tag="tr")
                nc.tensor.transpose(tr[:], h_bf[:, m * 128:(m + 1) * 128], ident[:])
                # chunks c = 2j+m of h^T for step t -> ring slot s
                nc.vector.tensor_copy(
                    ring_v[:, s, :, m, (t % 4) * 32:(t % 4) * 32 + 32],
                    tr[:].rearrange("p (j b) -> p j b", j=4),
                )
            if hm is not None:
                head_mms(hm[0], hm[1], range(6, KH), hm[2], hm[3])
                head_evac(hm[0], hm[1], hm[2], hm[3])

        # last token tile (steps 124..127) drains after the loop
        for pair in range(4):
            ps0 = hpsum.tile([128, NV], dt.float32, tag="hp")
            ps1 = hpsum.tile([128, NV], dt.float32, tag="hp")
            head_mms(MT_HEAD - 1, pair, range(KH), ps0, ps1)
            head_evac(MT_HEAD - 1, pair, ps0, ps1)


def prep_in_maps(z, x, W_h, b_h, emb, W_ih, W_hh, b_ih, b_hh, W_out, b_out):
    f32 = np.float32
    z = np.asarray(z, f32)
    W_h = np.asarray(W_h, f32)
    b_h = np.asarray(b_h, f32)
    emb = np.asarray(emb, f32)
    W_ih = np.asarray(W_ih, f32)
    W_hh = np.asarray(W_hh, f32)
    b_ih = np.asarray(b_ih, f32)
    b_hh = np.asarray(b_hh, f32)
    W_out = np.asarray(W_out, f32)
    b_out = np.asarray(b_out, f32)
    x = np.asarray(x)

    h0 = np.tanh(z @ W_h.T + b_h)                       # [B, H]
    e = emb[x]                                          # [B, T, D]
    # x_gates in torch column order, then permuted to the psum layout
    xg = e.reshape(-1, D_EMB) @ W_ih.T + (b_ih + b_hh)  # [B*T, 4H]
    xg = xg.reshape(B, T, 4, 4, 256)                    # [b,t,gate,j,u]
    # partition (j,b), free (t, gate, u)
    xgp = np.ascontiguousarray(xg.transpose(3, 0, 1, 2, 4)).reshape(128, T * 1024)

    # h0T[p, k*32+b] = h0[b, k*128+p]
    h0T = np.ascontiguousarray(h0.T.reshape(KH, 128, B).transpose(1, 0, 2)).reshape(128, KH * B)
    perm = _perm()
    Wp = W_hh.T[:, perm]                                # [H, 4H]
    Ws = np.ascontiguousarray(Wp.reshape(KH, 128, GW).transpose(1, 0, 2)).reshape(128, KH * GW)
    ident = np.eye(128, dtype=BF16)

    base = {
        "xg": xgp.astype(BF16),
        "h0T": h0T.astype(BF16),
        "Ws": Ws.astype(BF16),
        "ident": ident,
    }
    in_maps = []
    for c in range(N_CORES):
        Wsh = W_out[c * VSH:(c + 1) * VSH]              # [4000, 1024]
        WoT = np.ascontiguousarray(
            Wsh.reshape(NT_HEAD, NV, KH, 128).transpose(3, 0, 2, 1)
        ).reshape(128, NT_HEAD * KH * NV)
        bsh = b_out[c * VSH:(c + 1) * VSH]
        bias_o = np.ascontiguousarray(np.broadcast_to(bsh, (128, VSH)))
        m = dict(base)
        m["WoT"] = WoT.astype(BF16)
        m["bias_o"] = bias_o.astype(BF16)
        in_maps.append(m)
    return in_maps


def assemble(results):
    outs = [np.asarray(r["out"]).astype(np.float32).reshape(T, B, VSH) for r in results]
    full = np.concatenate(outs, axis=2)                 # [T, B, VOCAB]
    return np.ascontiguousarray(full.transpose(1, 0, 2))


def kernel(**inputs):
    in_maps = prep_in_maps(**inputs)
    nc = _build()
    res = bass_utils.run_bass_kernel_spmd(nc, in_maps, core_ids=list(range(N_CORES)))
    return assemble(res.results)


# revision 15
# speedup vs baseline: 1.3782x; 1.3782x over previous
"""LSTM decoder (nn_Decoder) on 8 Trainium2 NeuronCores.

Strategy (v2):
  - Replicate the sequential LSTM recurrence on all 8 cores (serial in T;
    B=32 gives too little parallelism to shard), shard the output head over
    the vocab dim: core c computes logits[:, :, c*4000:(c+1)*4000].
    Unshard = host-side concat; no device collectives.
  - Input-side gate projections x_gates = e @ W_ih^T + b_ih + b_hh are
    precomputed on HOST (17 GFLOP of the 320 total) and DMA-streamed to SBUF
    as bf16 in the PSUM gate layout. Each step they are injected into the
    PSUM accumulation with 32x32-identity matmuls at tile_position
    (32j, 32j) -- 2 matmul slots/step instead of the 10 the on-device
    embedding projection + rank-1 bias used to take.
  - Recurrence matmuls: gates[32,4096] via stationary h^T [128,32]
    replicated into the PE array's 4 column groups (tile_position=(0,32j)).
    Gate layout on PSUM: partition = 32j+b, free = gate*256+u.
  - h returned to h^T layout by 2 PE transposes into a 4-slot rolling ring
    (only 16 steps of h^T retained), freeing SBUF so that ALL head weights
    (8 vocab tiles x 8 K x 500) stay resident.
  - The vocab head is INTERLEAVED into the recurrence: token tile m
    (=steps 4m..4m+3, 128 tokens) is consumed during steps 4m+4..4m+7,
    one nt-pair (2 x 8 K matmuls, N=500) per step, filling the PE gaps
    left by the elementwise tail. Only tile 31 spills past the loop.
  - All matmuls bf16 inputs / fp32 PSUM; c, gates elementwise in fp32.

Host does data movement / layout prep, the h0 projection, the embedding
gather and the x_gates projection.
"""

import numpy as np
import ml_dtypes
from contextlib import ExitStack

import concourse.bass as bass  # noqa: F401
import concourse.tile as tile
import concourse.bacc as bacc
import concourse.mybir as mybir
from concourse import bass_utils

BF16 = ml_dtypes.bfloat16
N_CORES = 8
B, T = 32, 128
VOCAB, D_EMB, Z_DIM, HID = 32000, 512, 256, 1024
VSH = VOCAB // N_CORES    # 4000 vocab per core
NTOK = B * T              # 4096 tokens; token index = t*32 + b
KH = HID // 128           # 8 hidden K-chunks
GW = 4 * HID              # 4096 gate width
NT_HEAD = 8               # vocab tiles per core in the head
NV = VSH // NT_HEAD       # 500
MT_HEAD = NTOK // 128     # 32 token tiles in the head
CH = 8                    # steps per x_gates prefetch chunk
KORD = (0, 2, 4, 6, 1, 3, 5, 7)  # even h^T chunks first (they land first)

_NC_CACHE = {}


def _perm():
    # psum gate order n = j*1024 + gate*256 + u  ->  torch W column gate*1024 + j*256 + u
    j = np.arange(4)[:, None, None]
    gate = np.arange(4)[None, :, None]
    u = np.arange(256)[None, None, :]
    return (gate * 1024 + j * 256 + u).reshape(-1)


def _build(repeat=1):
    if repeat in _NC_CACHE:
        return _NC_CACHE[repeat]
    nc = bacc.Bacc("TRN2", debug=False, num_devices=N_CORES)
    dt = mybir.dt
    xg_d = nc.dram_tensor("xg", [128, T * 1024], dt.bfloat16, kind="ExternalInput").ap()
    h0T_d = nc.dram_tensor("h0T", [128, KH * B], dt.bfloat16, kind="ExternalInput").ap()
    Ws_d = nc.dram_tensor("Ws", [128, KH * GW], dt.bfloat16, kind="ExternalInput").ap()
    id_d = nc.dram_tensor("ident", [128, 128], dt.bfloat16, kind="ExternalInput").ap()
    id32_d = nc.dram_tensor("id32", [128, 32], dt.bfloat16, kind="ExternalInput").ap()
    WoT_d = nc.dram_tensor("WoT", [128, NT_HEAD * KH * NV], dt.bfloat16, kind="ExternalInput").ap()
    biaso_d = nc.dram_tensor("bias_o", [128, VSH], dt.bfloat16, kind="ExternalInput").ap()
    out_d = nc.dram_tensor("out", [NTOK, VSH], dt.bfloat16, kind="ExternalOutput").ap()

    with tile.TileContext(nc) as tc, ExitStack() as ctx:
        pers = ctx.enter_context(tc.tile_pool(name="pers", bufs=1))
        ident = pers.tile([128, 128], dt.bfloat16)
        nc.sync.dma_start(ident[:], id_d)
        id32 = pers.tile([128, 32], dt.bfloat16)
        nc.sync.dma_start(id32[:], id32_d)

        for _rep in range(repeat):
            _emit_body(nc, tc, ident, id32,
                       xg_d, h0T_d, Ws_d, WoT_d, biaso_d, out_d)
    nc.compile()
    _NC_CACHE[repeat] = nc
    return nc


def _emit_body(nc, tc, ident, id32, xg_d, h0T_d, Ws_d, WoT_d, biaso_d, out_d):
    dt = mybir.dt
    ACT = mybir.ActivationFunctionType
    with ExitStack() as rctx:
        wpool = rctx.enter_context(tc.tile_pool(name="ws", bufs=1))
        h0T_s = wpool.tile([128, KH * B], dt.bfloat16)
        nc.sync.dma_start(h0T_s[:], h0T_d)
        ws = wpool.tile([128, KH * GW], dt.bfloat16)    # 64KB/part
        for q in range(4):
            nc.sync.dma_start(ws[:, q * 2 * GW:(q + 1) * 2 * GW],
                              Ws_d[:, q * 2 * GW:(q + 1) * 2 * GW])
        wo = wpool.tile([128, NT_HEAD * KH * NV], dt.bfloat16)  # 64KB/part
        wostride = 2 * KH * NV
        for q in range(4):
            nc.sync.dma_start(wo[:, q * wostride:(q + 1) * wostride],
                              WoT_d[:, q * wostride:(q + 1) * wostride])
        bias_o = wpool.tile([128, VSH], dt.bfloat16)    # 8KB/part
        nc.sync.dma_start(bias_o[:], biaso_d)
        # rolling h^T ring: 4 slots x [KH x 128 tokens]; 8KB/part.
        # chunk index k = 2j+m laid out as (j, m) so the transpose-tail copy
        # can write the 4 even (m=0) / odd (m=1) chunks in one strided op.
        ring = wpool.tile([128, 4 * KH * 128], dt.bfloat16)
        ring_v = ring[:].rearrange("p (s j m tb) -> p s j m tb", s=4, j=4, m=2)

        xgpool = rctx.enter_context(tc.tile_pool(name="xg", bufs=2))
        gpsum = rctx.enter_context(tc.tile_pool(name="gps", bufs=2, space="PSUM"))
        hpsum = rctx.enter_context(tc.tile_pool(name="hps", bufs=2, space="PSUM"))
        tpsum = rctx.enter_context(tc.tile_pool(name="tps", bufs=2, space="PSUM"))
        ew = rctx.enter_context(tc.tile_pool(name="ew", bufs=2))
        opool = rctx.enter_context(tc.tile_pool(name="osb", bufs=4))
        cpool = rctx.enter_context(tc.tile_pool(name="cst", bufs=1))

        c_sb = cpool.tile([128, 256], dt.float32)
        nc.vector.memset(c_sb[:], 0.0)

        def inject(g, xgt, lo, hi):
            # PSUM init: g[32j+b, lo:hi] = x_gates chunk, via 32x32 identity
            # matmuls in row-group j / col-group j (start=True opens the
            # accumulation group; h matmuls then accumulate on top).
            for j in range(4):
                nc.tensor.matmul(
                    g[32 * j:32 * j + 32, lo:hi],
                    id32[32 * j:32 * j + 32, :],
                    xgt[32 * j:32 * j + 32, lo:hi],
                    start=True, stop=False,
                    tile_position=(32 * j, 32 * j),
                )

        def hblock(g, lhsT, k, lo, hi, start, stop):
            for j in range(4):
                nc.tensor.matmul(
                    g[32 * j:32 * j + 32, lo:hi],
                    lhsT,
                    ws[:, k * GW + j * 1024 + lo:k * GW + j * 1024 + hi],
                    start=start, stop=stop,
                    tile_position=(0, 32 * j),
                )

        def head_mms(m, pair, ks, ps0, ps1):
            # head matmuls (K-chunk subset ks) for token tile m, nt-pair
            s = m % 4
            for k in ks:
                lhsT = ring_v[:, s, k // 2, k % 2, :]
                for ps, nt in ((ps0, 2 * pair), (ps1, 2 * pair + 1)):
                    nc.tensor.matmul(
                        ps[:], lhsT,
                        wo[:, (nt * KH + k) * NV:(nt * KH + k + 1) * NV],
                        start=(k == 0), stop=(k == KH - 1),
                    )

        def head_evac(m, pair, ps0, ps1):
            for ps, nt in ((ps0, 2 * pair), (ps1, 2 * pair + 1)):
                osb = opool.tile([128, NV], dt.bfloat16, tag="osb")
                nc.vector.tensor_add(osb[:], ps[:], bias_o[:, nt * NV:(nt + 1) * NV])
                nc.sync.dma_start(
                    out_d[m * 128:(m + 1) * 128, nt * NV:(nt + 1) * NV], osb[:])

        xg_sb = None
        for t in range(T):
            if t % CH == 0:
                xg_sb = xgpool.tile([128, CH * 1024], dt.bfloat16, tag="xg")
                nc.sync.dma_start(xg_sb[:], xg_d[:, t * 1024:(t + CH) * 1024])
            xgt = xg_sb[:, (t % CH) * 1024:(t % CH + 1) * 1024]
            g = gpsum.tile([128, 1024], dt.float32, tag="g")

            def lhs(k):
                if t == 0:
                    return h0T_s[:, k * 32:(k + 1) * 32]
                tp = t - 1
                return ring_v[:, (tp // 4) % 4, k // 2, k % 2,
                              (tp % 4) * 32:(tp % 4) * 32 + 32]

            # region A: i,f gates (psum bank 0)
            inject(g, xgt, 0, 512)
            for ki, k in enumerate(KORD):
                hblock(g, lhs(k), k, 0, 512, False, ki == KH - 1)
            # region B: g,o gates (psum bank 1)
            inject(g, xgt, 512, 1024)
            for ki, k in enumerate(KORD):
                hblock(g, lhs(k), k, 512, 1024, False, ki == KH - 1)

            # interleaved head (tile m = t//4 - 1, nt-pair = t%4): first 6
            # K-chunks cover the elementwise-tail latency, then the h^T
            # transposes slot in exactly when h_bf is ready, then the rest.
            hm = None
            if t >= 4:
                hps0 = hpsum.tile([128, NV], dt.float32, tag="hp", name="hps0")
                hps1 = hpsum.tile([128, NV], dt.float32, tag="hp", name="hps1")
                hm = (t // 4 - 1, t % 4, hps0, hps1)
                head_mms(hm[0], hm[1], range(0, 6), hm[2], hm[3])

            # elementwise tail
            if_sb = ew.tile([128, 512], dt.float32, tag="if")
            nc.scalar.activation(if_sb[:], g[:, 0:512], ACT.Sigmoid)
            gg_sb = ew.tile([128, 256], dt.float32, tag="gg")
            nc.scalar.activation(gg_sb[:], g[:, 512:768], ACT.Tanh)
            o_sb = ew.tile([128, 256], dt.float32, tag="o")
            nc.scalar.activation(o_sb[:], g[:, 768:1024], ACT.Sigmoid)
            nc.vector.tensor_mul(c_sb[:], c_sb[:], if_sb[:, 256:512])
            t1 = ew.tile([128, 256], dt.float32, tag="t1")
            nc.vector.tensor_mul(t1[:], if_sb[:, 0:256], gg_sb[:])
            nc.vector.tensor_add(c_sb[:], c_sb[:], t1[:])
            tc_sb = ew.tile([128, 256], dt.float32, tag="tc")
            nc.scalar.activation(tc_sb[:], c_sb[:], ACT.Tanh)
            h_bf = ew.tile([128, 256], dt.bfloat16, tag="h")
            nc.vector.tensor_mul(h_bf[:], o_sb[:], tc_sb[:])
            s = (t // 4) % 4
            for m in range(2):
                tr = tpsum.tile([128, 128], dt.bfloat16, tag="tr")
                nc.tensor.transpose(tr[:], h_bf[:, m * 128:(m + 1) * 128], ident[:])
                # chunks c = 2j+m of h^T for step t -> ring slot s
                nc.vector.tensor_copy(
                    ring_v[:, s, :, m, (t % 4) * 32:(t % 4) * 32 + 32],
                    tr[:].rearrange("p (j b) -> p j b", j=4),
                )
            if hm is not None:
                head_mms(hm[0], hm[1], range(6, KH), hm[2], hm[3])
                head_evac(hm[0], hm[1], hm[2], hm[3])

        # last token tile (steps 124..127) drains after the loop
        for pair in range(4):
            ps0 = hpsum.tile([128, NV], dt.float32, tag="hp")
            ps1 = hpsum.tile([128, NV], dt.float32, tag="hp")
            head_mms(MT_HEAD - 1, pair, range(KH), ps0, ps1)
            head_evac(MT_HEAD - 1, pair, ps0, ps1)


def prep_in_maps(z, x, W_h, b_h, emb, W_ih, W_hh, b_ih, b_hh, W_out, b_out):
    f32 = np.float32
    z = np.asarray(z, f32)
    W_h = np.asarray(W_h, f32)
    b_h = np.asarray(b_h, f32)
    emb = np.asarray(emb, f32)
    W_ih = np.asarray(W_ih, f32)
    W_hh = np.asarray(W_hh, f32)
    b_ih = np.asarray(b_ih, f32)
    b_hh = np.asarray(b_hh, f32)
    W_out = np.asarray(W_out, f32)
    b_out = np.asarray(b_out, f32)
    x = np.asarray(x)

    h0 = np.tanh(z @ W_h.T + b_h)                       # [B, H]
    e = emb[x]                                          # [B, T, D]
    # x_gates in torch column order, then permuted to the psum layout
    xg = e.reshape(-1, D_EMB) @ W_ih.T + (b_ih + b_hh)  # [B*T, 4H]
    xg = xg.reshape(B, T, 4, 4, 256)                    # [b,t,gate,j,u]
    # partition (j,b), free (t, gate, u)
    xgp = np.ascontiguousarray(xg.transpose(3, 0, 1, 2, 4)).reshape(128, T * 1024)

    # h0T[p, k*32+b] = h0[b, k*128+p]
    h0T = np.ascontiguousarray(h0.T.reshape(KH, 128, B).transpose(1, 0, 2)).reshape(128, KH * B)
    perm = _perm()
    Wp = W_hh.T[:, perm]                                # [H, 4H]
    Ws = np.ascontiguousarray(Wp.reshape(KH, 128, GW).transpose(1, 0, 2)).reshape(128, KH * GW)
    ident = np.eye(128, dtype=BF16)
    id32 = np.tile(np.eye(32, dtype=BF16), (4, 1))      # [128, 32] block-diag rows

    base = {
        "xg": xgp.astype(BF16),
        "h0T": h0T.astype(BF16),
        "Ws": Ws.astype(BF16),
        "ident": ident,
        "id32": np.ascontiguousarray(id32),
    }
    in_maps = []
    for c in range(N_CORES):
        Wsh = W_out[c * VSH:(c + 1) * VSH]              # [4000, 1024]
        WoT = np.ascontiguousarray(
            Wsh.reshape(NT_HEAD, NV, KH, 128).transpose(3, 0, 2, 1)
        ).reshape(128, NT_HEAD * KH * NV)
        bsh = b_out[c * VSH:(c + 1) * VSH]
        bias_o = np.ascontiguousarray(np.broadcast_to(bsh, (128, VSH)))
        m = dict(base)
        m["WoT"] = WoT.astype(BF16)
        m["bias_o"] = bias_o.astype(BF16)
        in_maps.append(m)
    return in_maps


def assemble(results):
    outs = [np.asarray(r["out"]).astype(np.float32).reshape(T, B, VSH) for r in results]
    full = np.concatenate(outs, axis=2)                 # [T, B, VOCAB]
    return np.ascontiguousarray(full.transpose(1, 0, 2))


def kernel(**inputs):
    in_maps = prep_in_maps(**inputs)
    nc = _build()
    res = bass_utils.run_bass_kernel_spmd(nc, in_maps, core_ids=list(range(N_CORES)))
    return assemble(res.results)


# revision 16
# speedup vs baseline: 2.0146x; 1.4618x over previous
"""LSTM decoder (nn_Decoder) on 8 Trainium2 NeuronCores.

Strategy (v2):
  - Replicate the sequential LSTM recurrence on all 8 cores (serial in T;
    B=32 gives too little parallelism to shard), shard the output head over
    the vocab dim: core c computes logits[:, :, c*4000:(c+1)*4000].
    Unshard = host-side concat; no device collectives.
  - Input-side gate projections x_gates = e @ W_ih^T + b_ih + b_hh are
    precomputed on HOST (17 GFLOP of the 320 total) and DMA-streamed to SBUF
    as bf16 in the PSUM gate layout. Each step they are injected into the
    PSUM accumulation with 32x32-identity matmuls at tile_position
    (32j, 32j) -- 2 matmul slots/step instead of the 10 the on-device
    embedding projection + rank-1 bias used to take.
  - Recurrence matmuls: gates[32,4096] via stationary h^T [128,32]
    replicated into the PE array's 4 column groups (tile_position=(0,32j)).
    Gate layout on PSUM: partition = 32j+b, free = gate*256+u.
  - h returned to h^T layout by 2 PE transposes into a 4-slot rolling ring
    (only 16 steps of h^T retained), freeing SBUF so that ALL head weights
    (8 vocab tiles x 8 K x 500) stay resident.
  - The vocab head is INTERLEAVED into the recurrence: token tile m
    (=steps 4m..4m+3, 128 tokens) is consumed during steps 4m+4..4m+7,
    one nt-pair (2 x 8 K matmuls, N=500) per step, filling the PE gaps
    left by the elementwise tail. Only tile 31 spills past the loop.
  - All matmuls bf16 inputs / fp32 PSUM; c, gates elementwise in fp32.

Host does data movement / layout prep, the h0 projection, the embedding
gather and the x_gates projection.
"""

import numpy as np
import ml_dtypes
from contextlib import ExitStack

import concourse.bass as bass  # noqa: F401
import concourse.tile as tile
import concourse.bacc as bacc
import concourse.mybir as mybir
from concourse import bass_utils

BF16 = ml_dtypes.bfloat16
N_CORES = 8
B, T = 32, 128
VOCAB, D_EMB, Z_DIM, HID = 32000, 512, 256, 1024
VSH = VOCAB // N_CORES    # 4000 vocab per core
NTOK = B * T              # 4096 tokens; token index = t*32 + b
KH = HID // 128           # 8 hidden K-chunks
GW = 4 * HID              # 4096 gate width
NT_HEAD = 8               # vocab tiles per core in the head
NV = VSH // NT_HEAD       # 500
MT_HEAD = NTOK // 128     # 32 token tiles in the head
CH = 8                    # steps per x_gates prefetch chunk
KORD = (0, 2, 4, 6, 1, 3, 5, 7)  # even h^T chunks first (they land first)

_NC_CACHE = {}


def _perm():
    # psum gate order n = j*1024 + gate*256 + u  ->  torch W column gate*1024 + j*256 + u
    j = np.arange(4)[:, None, None]
    gate = np.arange(4)[None, :, None]
    u = np.arange(256)[None, None, :]
    return (gate * 1024 + j * 256 + u).reshape(-1)


def _build(repeat=1):
    if repeat in _NC_CACHE:
        return _NC_CACHE[repeat]
    nc = bacc.Bacc("TRN2", debug=False, num_devices=N_CORES)
    dt = mybir.dt
    xg_d = nc.dram_tensor("xg", [128, T * 1024], dt.bfloat16, kind="ExternalInput").ap()
    h0T_d = nc.dram_tensor("h0T", [128, KH * B], dt.bfloat16, kind="ExternalInput").ap()
    Ws_d = nc.dram_tensor("Ws", [128, KH * GW], dt.bfloat16, kind="ExternalInput").ap()
    id_d = nc.dram_tensor("ident", [128, 128], dt.bfloat16, kind="ExternalInput").ap()
    id32_d = nc.dram_tensor("id32", [128, 32], dt.bfloat16, kind="ExternalInput").ap()
    WoT_d = nc.dram_tensor("WoT", [128, NT_HEAD * KH * NV], dt.bfloat16, kind="ExternalInput").ap()
    biaso_d = nc.dram_tensor("bias_o", [128, VSH], dt.bfloat16, kind="ExternalInput").ap()
    out_d = nc.dram_tensor("out", [NTOK, VSH], dt.bfloat16, kind="ExternalOutput").ap()

    with tile.TileContext(nc) as tc, ExitStack() as ctx:
        pers = ctx.enter_context(tc.tile_pool(name="pers", bufs=1))
        ident = pers.tile([128, 128], dt.bfloat16)
        nc.sync.dma_start(ident[:], id_d)
        id32 = pers.tile([128, 32], dt.bfloat16)
        nc.sync.dma_start(id32[:], id32_d)

        for _rep in range(repeat):
            _emit_body(nc, tc, ident, id32,
                       xg_d, h0T_d, Ws_d, WoT_d, biaso_d, out_d)
    nc.compile()
    _NC_CACHE[repeat] = nc
    return nc


def _emit_body(nc, tc, ident, id32, xg_d, h0T_d, Ws_d, WoT_d, biaso_d, out_d):
    dt = mybir.dt
    ACT = mybir.ActivationFunctionType
    with ExitStack() as rctx:
        wpool = rctx.enter_context(tc.tile_pool(name="ws", bufs=1))
        h0T_s = wpool.tile([128, KH * B], dt.bfloat16)
        nc.sync.dma_start(h0T_s[:], h0T_d)
        ws = wpool.tile([128, KH * GW], dt.bfloat16)    # 64KB/part
        for q in range(4):
            nc.sync.dma_start(ws[:, q * 2 * GW:(q + 1) * 2 * GW],
                              Ws_d[:, q * 2 * GW:(q + 1) * 2 * GW])
        wo = wpool.tile([128, NT_HEAD * KH * NV], dt.bfloat16)  # 64KB/part
        wostride = 2 * KH * NV
        for q in range(4):
            nc.sync.dma_start(wo[:, q * wostride:(q + 1) * wostride],
                              WoT_d[:, q * wostride:(q + 1) * wostride])
        bias_o = wpool.tile([128, VSH], dt.bfloat16)    # 8KB/part
        nc.sync.dma_start(bias_o[:], biaso_d)
        # rolling h^T ring: 4 slots x [KH x 128 tokens]; 8KB/part.
        # chunk index k = 2j+m laid out as (j, m) so the transpose-tail copy
        # can write the 4 even (m=0) / odd (m=1) chunks in one strided op.
        ring = wpool.tile([128, 4 * KH * 128], dt.bfloat16)
        ring_v = ring[:].rearrange("p (s j m tb) -> p s j m tb", s=4, j=4, m=2)

        xgpool = rctx.enter_context(tc.tile_pool(name="xg", bufs=2))
        gpsum = rctx.enter_context(tc.tile_pool(name="gps", bufs=2, space="PSUM"))
        hpsum = rctx.enter_context(tc.tile_pool(name="hps", bufs=2, space="PSUM"))
        tpsum = rctx.enter_context(tc.tile_pool(name="tps", bufs=2, space="PSUM"))
        ew = rctx.enter_context(tc.tile_pool(name="ew", bufs=2))
        opool = rctx.enter_context(tc.tile_pool(name="osb", bufs=4))
        cpool = rctx.enter_context(tc.tile_pool(name="cst", bufs=1))

        c_sb = cpool.tile([128, 256], dt.float32)
        nc.vector.memset(c_sb[:], 0.0)

        def inject(g, xgt, lo, hi):
            # PSUM init: g[32j+b, lo:hi] = x_gates chunk, via 32x32 identity
            # matmuls in row-group j / col-group j (start=True opens the
            # accumulation group; h matmuls then accumulate on top).
            for j in range(4):
                nc.tensor.matmul(
                    g[32 * j:32 * j + 32, lo:hi],
                    id32[32 * j:32 * j + 32, :],
                    xgt[32 * j:32 * j + 32, lo:hi],
                    start=True, stop=False,
                    tile_position=(32 * j, 32 * j),
                )

        def hblock(g, lhsT, k, lo, hi, start, stop):
            for j in range(4):
                nc.tensor.matmul(
                    g[32 * j:32 * j + 32, lo:hi],
                    lhsT,
                    ws[:, k * GW + j * 1024 + lo:k * GW + j * 1024 + hi],
                    start=start, stop=stop,
                    tile_position=(0, 32 * j),
                )

        def head_mms(m, pair, ks, ps0, ps1):
            # head matmuls (K-chunk subset ks) for token tile m, nt-pair
            s = m % 4
            for k in ks:
                lhsT = ring_v[:, s, k // 2, k % 2, :]
                for ps, nt in ((ps0, 2 * pair), (ps1, 2 * pair + 1)):
                    nc.tensor.matmul(
                        ps[:], lhsT,
                        wo[:, (nt * KH + k) * NV:(nt * KH + k + 1) * NV],
                        start=(k == 0), stop=(k == KH - 1),
                    )

        def head_evac(m, pair, ps0, ps1):
            for ps, nt in ((ps0, 2 * pair), (ps1, 2 * pair + 1)):
                osb = opool.tile([128, NV], dt.bfloat16, tag="osb")
                nc.vector.tensor_add(osb[:], ps[:], bias_o[:, nt * NV:(nt + 1) * NV])
                nc.sync.dma_start(
                    out_d[m * 128:(m + 1) * 128, nt * NV:(nt + 1) * NV], osb[:])

        # xg chunk tiles; chunk c covers steps [CH*c, CH*c+CH)
        xg_tiles = {}

        def xg_fetch(c):
            xt = xgpool.tile([128, CH * 1024], dt.bfloat16, tag="xg", name="xgc")
            nc.sync.dma_start(xt[:], xg_d[:, c * CH * 1024:(c + 1) * CH * 1024])
            xg_tiles[c] = xt

        def xg_at(t):
            return xg_tiles[t // CH][:, (t % CH) * 1024:(t % CH + 1) * 1024]

        xg_fetch(0)
        g_cur = None
        for t in range(T):
            # prefetch the next xg chunk mid-way through the current one
            if t % CH == 4 and t // CH + 1 < T // CH:
                xg_fetch(t // CH + 1)
            if t == 0:
                g_cur = gpsum.tile([128, 1024], dt.float32, tag="g", name="g0")
                inject(g_cur, xg_at(0), 0, 512)
                inject(g_cur, xg_at(0), 512, 1024)
            g = g_cur

            def lhs(k):
                if t == 0:
                    return h0T_s[:, k * 32:(k + 1) * 32]
                tp = t - 1
                return ring_v[:, (tp // 4) % 4, k // 2, k % 2,
                              (tp % 4) * 32:(tp % 4) * 32 + 32]

            # region A: i,f gates (psum bank 0); inject already seeded
            for ki, k in enumerate(KORD):
                hblock(g, lhs(k), k, 0, 512, False, ki == KH - 1)
            # region B: g,o gates (psum bank 1)
            for ki, k in enumerate(KORD):
                hblock(g, lhs(k), k, 512, 1024, False, ki == KH - 1)
            # seed step t+1's x_gates into the other psum buffer now —
            # independent of h_t, fills the PE gap before the head burst
            if t + 1 < T:
                g_cur = gpsum.tile([128, 1024], dt.float32, tag="g", name="gn")
                inject(g_cur, xg_at(t + 1), 0, 512)
                inject(g_cur, xg_at(t + 1), 512, 1024)

            # interleaved head (tile m = t//4 - 1, nt-pair = t%4): first 6
            # K-chunks cover the elementwise-tail latency, then the h^T
            # transposes slot in exactly when h_bf is ready, then the rest.
            hm = None
            if t >= 4:
                hps0 = hpsum.tile([128, NV], dt.float32, tag="hp", name="hps0")
                hps1 = hpsum.tile([128, NV], dt.float32, tag="hp", name="hps1")
                hm = (t // 4 - 1, t % 4, hps0, hps1)
                head_mms(hm[0], hm[1], range(0, 6), hm[2], hm[3])

            # elementwise tail
            if_sb = ew.tile([128, 512], dt.float32, tag="if")
            nc.scalar.activation(if_sb[:], g[:, 0:512], ACT.Sigmoid)
            gg_sb = ew.tile([128, 256], dt.float32, tag="gg")
            nc.scalar.activation(gg_sb[:], g[:, 512:768], ACT.Tanh)
            o_sb = ew.tile([128, 256], dt.float32, tag="o")
            nc.scalar.activation(o_sb[:], g[:, 768:1024], ACT.Sigmoid)
            nc.vector.tensor_mul(c_sb[:], c_sb[:], if_sb[:, 256:512])
            t1 = ew.tile([128, 256], dt.float32, tag="t1")
            nc.vector.tensor_mul(t1[:], if_sb[:, 0:256], gg_sb[:])
            nc.vector.tensor_add(c_sb[:], c_sb[:], t1[:])
            tc_sb = ew.tile([128, 256], dt.float32, tag="tc")
            nc.scalar.activation(tc_sb[:], c_sb[:], ACT.Tanh)
            h_bf = ew.tile([128, 256], dt.bfloat16, tag="h")
            nc.vector.tensor_mul(h_bf[:], o_sb[:], tc_sb[:])
            s = (t // 4) % 4
            for m in range(2):
                tr = tpsum.tile([128, 128], dt.bfloat16, tag="tr")
                nc.tensor.transpose(tr[:], h_bf[:, m * 128:(m + 1) * 128], ident[:])
                # chunks c = 2j+m of h^T for step t -> ring slot s
                nc.vector.tensor_copy(
                    ring_v[:, s, :, m, (t % 4) * 32:(t % 4) * 32 + 32],
                    tr[:].rearrange("p (j b) -> p j b", j=4),
                )
            if hm is not None:
                head_mms(hm[0], hm[1], range(6, KH), hm[2], hm[3])
                head_evac(hm[0], hm[1], hm[2], hm[3])

        # last token tile (steps 124..127) drains after the loop
        for pair in range(4):
            ps0 = hpsum.tile([128, NV], dt.float32, tag="hp")
            ps1 = hpsum.tile([128, NV], dt.float32, tag="hp")
            head_mms(MT_HEAD - 1, pair, range(KH), ps0, ps1)
            head_evac(MT_HEAD - 1, pair, ps0, ps1)


def prep_in_maps(z, x, W_h, b_h, emb, W_ih, W_hh, b_ih, b_hh, W_out, b_out):
    f32 = np.float32
    z = np.asarray(z, f32)
    W_h = np.asarray(W_h, f32)
    b_h = np.asarray(b_h, f32)
    emb = np.asarray(emb, f32)
    W_ih = np.asarray(W_ih, f32)
    W_hh = np.asarray(W_hh, f32)
    b_ih = np.asarray(b_ih, f32)
    b_hh = np.asarray(b_hh, f32)
    W_out = np.asarray(W_out, f32)
    b_out = np.asarray(b_out, f32)
    x = np.asarray(x)

    h0 = np.tanh(z @ W_h.T + b_h)                       # [B, H]
    e = emb[x]                                          # [B, T, D]
    # x_gates in torch column order, then permuted to the psum layout
    xg = e.reshape(-1, D_EMB) @ W_ih.T + (b_ih + b_hh)  # [B*T, 4H]
    xg = xg.reshape(B, T, 4, 4, 256)                    # [b,t,gate,j,u]
    # partition (j,b), free (t, gate, u)
    xgp = np.ascontiguousarray(xg.transpose(3, 0, 1, 2, 4)).reshape(128, T * 1024)

    # h0T[p, k*32+b] = h0[b, k*128+p]
    h0T = np.ascontiguousarray(h0.T.reshape(KH, 128, B).transpose(1, 0, 2)).reshape(128, KH * B)
    perm = _perm()
    Wp = W_hh.T[:, perm]                                # [H, 4H]
    Ws = np.ascontiguousarray(Wp.reshape(KH, 128, GW).transpose(1, 0, 2)).reshape(128, KH * GW)
    ident = np.eye(128, dtype=BF16)
    id32 = np.tile(np.eye(32, dtype=BF16), (4, 1))      # [128, 32] block-diag rows

    base = {
        "xg": xgp.astype(BF16),
        "h0T": h0T.astype(BF16),
        "Ws": Ws.astype(BF16),
        "ident": ident,
        "id32": np.ascontiguousarray(id32),
    }
    in_maps = []
    for c in range(N_CORES):
        Wsh = W_out[c * VSH:(c + 1) * VSH]              # [4000, 1024]
        WoT = np.ascontiguousarray(
            Wsh.reshape(NT_HEAD, NV, KH, 128).transpose(3, 0, 2, 1)
        ).reshape(128, NT_HEAD * KH * NV)
        bsh = b_out[c * VSH:(c + 1) * VSH]
        bias_o = np.ascontiguousarray(np.broadcast_to(bsh, (128, VSH)))
        m = dict(base)
        m["WoT"] = WoT.astype(BF16)
        m["bias_o"] = bias_o.astype(BF16)
        in_maps.append(m)
    return in_maps


def assemble(results):
    outs = [np.asarray(r["out"]).astype(np.float32).reshape(T, B, VSH) for r in results]
    full = np.concatenate(outs, axis=2)                 # [T, B, VOCAB]
    return np.ascontiguousarray(full.transpose(1, 0, 2))


def kernel(**inputs):
    in_maps = prep_in_maps(**inputs)
    nc = _build()
    res = bass_utils.run_bass_kernel_spmd(nc, in_maps, core_ids=list(range(N_CORES)))
    return assemble(res.results)
